# revision 1
# baseline (speedup 1.0000x reference)
"""GCN 2-layer decoder on 8 trn2 NeuronCores.

Algorithm (per core, nodes dest-sharded):
  deg[c]  = sum of in-edge weights (+1 self loop)   [host pads slots, DVE reduce]
  dinv    = 1/sqrt(deg)
  xt1[r]  = dinv[r] * (z @ W1)[r]      -> bf16 rows in a Shared DRAM table
  agg[c]  = sum_e ew_e * xt1[row_e]    [dma_gather rows + selector-matmul in PSUM]
  h1s[c]  = relu(dinv[c]*agg[c] + b1) * dinv[c]
  xt2[r]  = (h1s @ W2)[r]              -> bf16 rows in Shared table
  out[c]  = dinv[c] * (sum_e ew_e * xt2[row_e]) + b2

Edges are sorted by (dest-half, source-quarter, dest-block); each
(half, quarter, block) run is padded to a uniform (cross-core) tile count so
the single SPMD program works for all 8 cores.  Source rows are fetched with
gpsimd.dma_gather (int16 quarter-local indices); per 128-edge tile a [128,128]
bf16 selector S (S[e,d] = ew_e * (d == dloc_e%128)) is built with one DVE
tensor_scalar and PE accumulates S.T @ G into the block's PSUM column.
"""

import math
from contextlib import ExitStack
from dataclasses import dataclass

import numpy as np

P = 128


@dataclass(frozen=True)
class Cfg:
    n: int              # total nodes
    ncores: int         # 8
    qn: int             # source quarters (index range per gather table slice)
    f_in: int           # 64
    f_hid: int          # 64
    f_out: int          # 32
    ch_tiles: int = 32  # gather chunk size in 128-edge tiles

    @property
    def nshard(self):
        return self.n // self.ncores

    @property
    def nblk(self):
        return math.ceil(self.nshard / P)

    @property
    def nblk_h(self):
        return math.ceil(self.nblk / 2)

    @property
    def dests_pad(self):
        return self.nblk * P

    @property
    def qsize(self):
        return self.n // self.qn


FULL_CFG = Cfg(n=100000, ncores=8, qn=4, f_in=64, f_hid=64, f_out=32)


# ---------------------------------------------------------------- host side

def preprocess(cfg: Cfg, edge_index: np.ndarray, edge_attr: np.ndarray):
    """Build the uniform schedule + per-core device input arrays."""
    n = cfg.n
    ns = cfg.nshard
    nbh = cfg.nblk_h

    rows = np.concatenate([edge_index[0], np.arange(n, dtype=np.int64)])
    cols = np.concatenate([edge_index[1], np.arange(n, dtype=np.int64)])
    ews = np.concatenate([edge_attr.astype(np.float32),
                          np.ones(n, dtype=np.float32)])

    core = cols // ns
    dloc = (cols - core * ns).astype(np.int64)
    q = rows // cfg.qsize
    rloc = (rows - q * cfg.qsize).astype(np.int64)
    blk = dloc // P
    half = (blk >= nbh).astype(np.int64)
    bh = blk - half * nbh  # block within half

    assert rloc.max() < 32768, "quarter-local index must fit int16"

    # run id in schedule order: (half, quarter, block-in-half)
    run_id = (half * cfg.qn + q) * nbh + bh
    n_runs = 2 * cfg.qn * nbh

    # counts per (core, run)
    cnt = np.zeros((cfg.ncores, n_runs), dtype=np.int64)
    np.add.at(cnt, (core, run_id), 1)
    T = np.maximum(1, np.ceil(cnt.max(axis=0) / P).astype(np.int64))  # [n_runs]

    run_tile_off = np.concatenate([[0], np.cumsum(T)])   # tile offset per run
    total_tiles = int(run_tile_off[-1])                   # tiles per layer
    total_slots = total_tiles * P

    # per-run tile metadata (uniform across cores)
    tile_run = np.repeat(np.arange(n_runs), T)            # [total_tiles]
    t_half = tile_run // (cfg.qn * nbh)
    t_q = (tile_run // nbh) % cfg.qn
    t_bh = tile_run % nbh
    # j = tile index within run
    t_j = np.arange(total_tiles) - run_tile_off[tile_run]
    t_start = (t_q == 0) & (t_j == 0)
    last_j = T[tile_run] - 1
    t_stop = (t_q == cfg.qn - 1) & (t_j == last_j)

    # per-(half,q) segment boundaries in tile units
    seg_tiles = {}
    for h in range(2):
        for qq in range(cfg.qn):
            r0 = (h * cfg.qn + qq) * nbh
            seg_tiles[(h, qq)] = int(T[r0:r0 + nbh].sum())

    sched = {
        "T": T, "tile_run": tile_run, "t_half": t_half, "t_q": t_q,
        "t_bh": t_bh, "t_start": t_start, "t_stop": t_stop, "t_j": t_j,
        "run_tile_off": run_tile_off, "total_tiles": total_tiles,
        "seg_tiles": seg_tiles,
    }

    # degree slot count (uniform): max in-degree over all nodes
    deg_cnt = np.bincount(cols, minlength=n)  # includes self loops
    dslot = int(math.ceil((deg_cnt.max() + 1) / 8) * 8)
    sched["dslot"] = dslot

    per_core = []
    order_all = np.lexsort((dloc, run_id, core))  # sorted by core, run, dloc
    core_sorted = core[order_all]
    core_bounds = np.searchsorted(core_sorted, np.arange(cfg.ncores + 1))

    for c in range(cfg.ncores):
        sel = order_all[core_bounds[c]:core_bounds[c + 1]]
        c_run = run_id[sel]
        c_rloc = rloc[sel]
        c_dloc = dloc[sel]
        c_ew = ews[sel]

        # rank within run (sel is sorted by run)
        run_starts = np.searchsorted(c_run, np.arange(n_runs))
        rank = np.arange(len(sel)) - run_starts[c_run]
        slot = (run_tile_off[c_run] * P + rank).astype(np.int64)

        s_rloc = np.zeros(total_slots, dtype=np.int16)
        s_dlocrel = np.zeros(total_slots, dtype=np.float32)
        s_ew = np.zeros(total_slots, dtype=np.float32)
        s_rloc[slot] = c_rloc.astype(np.int16)
        s_dlocrel[slot] = (c_dloc % P).astype(np.float32)
        s_ew[slot] = c_ew

        # dloc/ew in [128, total_tiles] device layout
        dloc_col = np.ascontiguousarray(s_dlocrel.reshape(total_tiles, P).T)
        ew_col = np.ascontiguousarray(s_ew.reshape(total_tiles, P).T)

        # idx arrays per (h,q) segment, wrapped 16 + replicated to 128 parts
        idx_segs = {}
        t0 = 0
        for h in range(2):
            for qq in range(cfg.qn):
                st = seg_tiles[(h, qq)]
                seg = s_rloc[t0 * P:(t0 + st) * P]
                wrapped = np.ascontiguousarray(seg.reshape(-1, 16).T)  # [16, S/16]
                idx_segs[(h, qq)] = np.ascontiguousarray(
                    np.tile(wrapped, (P // 16, 1)))
                t0 += st

        # degree pad array [128, nblk*dslot]
        dmask = (cols // ns) == c
        dd = dloc[dmask]
        dw = ews[dmask]
        o2 = np.argsort(dd, kind="stable")
        dd, dw = dd[o2], dw[o2]
        dstart = np.searchsorted(dd, np.arange(ns))
        drank = np.arange(len(dd)) - dstart[dd]
        degpad = np.zeros((P, cfg.nblk * dslot), dtype=np.float32)
        degpad[dd % P, (dd // P) * dslot + drank] = dw
        # phantom dests get deg=1 to avoid 1/0
        for ph in range(ns, cfg.dests_pad):
            degpad[ph % P, (ph // P) * dslot] = 1.0

        per_core.append({
            "dloc_col": dloc_col, "ew_col": ew_col, "idx_segs": idx_segs,
            "degpad": degpad,
        })

    return sched, per_core


# ---------------------------------------------------------------- device side

def build_program(cfg: Cfg, sched, dbg: bool = False):
    import ml_dtypes  # noqa: F401
    from concourse import bacc, bass, mybir, tile
    from concourse.library_config import mlp

    f32 = mybir.dt.float32
    bf16 = mybir.dt.bfloat16
    i16 = mybir.dt.int16
    Alu = mybir.AluOpType
    Act = mybir.ActivationFunctionType

    n, ns, nbh, nblk = cfg.n, cfg.nshard, cfg.nblk_h, cfg.nblk
    dslot = sched["dslot"]
    TT = sched["total_tiles"]
    f_in, f_hid, f_out = cfg.f_in, cfg.f_hid, cfg.f_out

    nc = bacc.Bacc("TRN2", target_bir_lowering=False, debug=False,
                   enable_asserts=False, num_devices=cfg.ncores)

    # ---- I/O declarations
    zT_d = nc.dram_tensor("zT", [f_in, ns], f32, kind="ExternalInput")
    w1_d = nc.dram_tensor("W1", [f_in, f_hid], f32, kind="ExternalInput")
    w2_d = nc.dram_tensor("W2", [f_hid, f_out], f32, kind="ExternalInput")
    b1b_d = nc.dram_tensor("b1b", [P, f_hid], f32, kind="ExternalInput")
    b2b_d = nc.dram_tensor("b2b", [P, f_out], f32, kind="ExternalInput")
    iota_d = nc.dram_tensor("iota", [P, P], bf16, kind="ExternalInput")
    ident_d = nc.dram_tensor("ident", [P, P], f32, kind="ExternalInput")
    degpad_d = nc.dram_tensor("degpad", [P, nblk * dslot], f32,
                              kind="ExternalInput")
    dloc_d = nc.dram_tensor("dloc", [P, TT], f32, kind="ExternalInput")
    ew_d = nc.dram_tensor("ew", [P, TT], f32, kind="ExternalInput")
    idx_d = {}
    for h in range(2):
        for qq in range(cfg.qn):
            st = sched["seg_tiles"][(h, qq)]
            idx_d[(h, qq)] = nc.dram_tensor(
                f"idx_h{h}q{qq}", [P, st * P // 16], i16, kind="ExternalInput")
    out_d = nc.dram_tensor("out", [cfg.dests_pad, f_out], f32,
                           kind="ExternalOutput")
    if dbg:
        dbg_dinv = nc.dram_tensor("dbg_dinv", [P, nblk], f32,
                                  kind="ExternalOutput")
        dbg_xg1 = nc.dram_tensor("dbg_xg1", [n, P], bf16,
                                 kind="ExternalOutput")
        dbg_agg1 = nc.dram_tensor("dbg_agg1", [P, nblk * f_hid], f32,
                                  kind="ExternalOutput")
        dbg_h1s = nc.dram_tensor("dbg_h1s", [P, nblk * f_hid], f32,
                                 kind="ExternalOutput")
        dbg_xg2 = nc.dram_tensor("dbg_xg2", [n, P], bf16,
                                 kind="ExternalOutput")

    # local slice + shared gathered tables (rows padded to 128 bf16 = 256B)
    xloc1 = nc.dram_tensor("xloc1", [ns, P], bf16, kind="Internal")
    xg1 = nc.dram_tensor("xg1", [n, P], bf16, kind="Internal",
                         addr_space="Shared")
    xloc2 = nc.dram_tensor("xloc2", [ns, P], bf16, kind="Internal")
    xg2 = nc.dram_tensor("xg2", [n, P], bf16, kind="Internal",
                         addr_space="Shared")

    groups = [list(range(cfg.ncores))]

    with tile.TileContext(nc, num_cores=cfg.ncores) as tc, \
            ExitStack() as ctx:
        nc.gpsimd.load_library(mlp)

        cpool = ctx.enter_context(tc.tile_pool(name="const", bufs=1))

        def load_const(dram, shape, dtype, tag):
            t = cpool.tile(shape, dtype, tag=tag)
            nc.sync.dma_start(out=t[:], in_=dram[:])
            return t

        iota_sb = load_const(iota_d, [P, P], bf16, "iota")
        ident_sb = load_const(ident_d, [P, P], f32, "ident")
        b1b_sb = load_const(b1b_d, [P, f_hid], f32, "b1b")
        b2b_sb = load_const(b2b_d, [P, f_out], f32, "b2b")
        w1_sb = load_const(w1_d, [f_in, f_hid], f32, "w1")
        w2_sb = load_const(w2_d, [f_hid, f_out], f32, "w2")
        dloc_sb = load_const(dloc_d, [P, TT], f32, "dloc")
        ew_sb = load_const(ew_d, [P, TT], f32, "ew")
        idx_sb = {}
        for h in range(2):
            for qq in range(cfg.qn):
                st = sched["seg_tiles"][(h, qq)]
                idx_sb[(h, qq)] = load_const(idx_d[(h, qq)],
                                             [P, st * P // 16], i16,
                                             f"idx{h}{qq}")

        # ---- deg -> dinv
        dinv_sb = cpool.tile([P, nblk], f32, tag="dinv")
        with tc.tile_pool(name="deg", bufs=1) as dpool:
            degpad_sb = dpool.tile([P, nblk * dslot], f32)
            nc.sync.dma_start(out=degpad_sb[:], in_=degpad_d[:])
            deg_sb = dpool.tile([P, nblk], f32)
            nc.vector.tensor_reduce(
                out=deg_sb[:],
                in_=degpad_sb[:].rearrange("p (b s) -> p b s", s=dslot),
                axis=mybir.AxisListType.X, op=Alu.add)
            rdeg_sb = dpool.tile([P, nblk], f32)
            nc.vector.reciprocal(out=rdeg_sb[:], in_=deg_sb[:])
            nc.scalar.activation(out=dinv_sb[:], in_=rdeg_sb[:], func=Act.Sqrt)
            if dbg:
                nc.sync.dma_start(out=dbg_dinv[:], in_=dinv_sb[:])

        # ---- xt1 = dinv * (z @ W1), written as bf16 rows of xloc1
        def emit_xt_prep(src_get, w_sb, fdim_in, fdim_out, xloc, scale):
            """src_get(chunk)->AP [fdim_in, width] feature-major source."""
            with tc.tile_pool(name="xprep", bufs=3) as xp, \
                    tc.tile_pool(name="xprep_ps", bufs=3, space="PSUM") as xps:
                nchunks = math.ceil(ns / 512)
                for ch in range(nchunks):
                    n0 = ch * 512
                    width = min(512, ns - n0)
                    ps_x = xps.tile([fdim_out, 512], f32, tag="ps_x")
                    nc.tensor.matmul(out=ps_x[:, :width], lhsT=w_sb[:],
                                     rhs=src_get(ch, width), start=True,
                                     stop=True)
                    xT = xp.tile([fdim_out, 512], f32, tag="xT")
                    nc.vector.tensor_copy(out=xT[:, :width], in_=ps_x[:, :width])
                    for j in range(math.ceil(width / P)):
                        nb = ch * 4 + j
                        w = min(P, width - j * P)
                        ps_t = xps.tile([P, fdim_out], f32, tag="ps_t")
                        nc.tensor.transpose(
                            out=ps_t[:w, :], in_=xT[:, j * P:j * P + w],
                            identity=ident_sb[:fdim_out, :fdim_out])
                        xb = xp.tile([P, fdim_out], bf16, tag="xb")
                        if scale:
                            nc.vector.tensor_scalar(
                                out=xb[:w, :], in0=ps_t[:w, :],
                                scalar1=dinv_sb[:w, nb:nb + 1], scalar2=None,
                                op0=Alu.mult)
                        else:
                            nc.vector.tensor_copy(out=xb[:w, :], in_=ps_t[:w, :])
                        nc.sync.dma_start(
                            out=xloc[n0 + j * P:n0 + j * P + w, 0:fdim_out],
                            in_=xb[:w, :])

        with tc.tile_pool(name="zt", bufs=1) as zpool:
            zT_sb = zpool.tile([f_in, ns], f32)
            nc.sync.dma_start(out=zT_sb[:], in_=zT_d[:])
            emit_xt_prep(lambda ch, w: zT_sb[:, ch * 512:ch * 512 + w],
                         w1_sb, f_in, f_hid, xloc1, scale=True)

        nc.gpsimd.collective_compute(
            "AllGather", Alu.bypass, replica_groups=groups,
            ins=[xloc1[:]], outs=[xg1[:]])
        if dbg:
            nc.sync.dma_start(out=dbg_xg1[:], in_=xg1[:])

        # ---- aggregation layer
        def emit_agg(xg, fdim, epilogue, pspool, accpool):
            gpool = ctx_pools["g"]
            spool = ctx_pools["s"]
            g_tile0 = 0
            for h in range(2):
                acc = accpool.tile([P, nbh * fdim], f32, tag="agg_acc")
                nc.vector.memset(acc[:], 0.0)
                for qq in range(cfg.qn):
                    ps_hq = pspool.tile([P, nbh * fdim], f32, tag="ps_hq")
                    st = sched["seg_tiles"][(h, qq)]
                    ixs = idx_sb[(h, qq)]
                    for c0 in range(0, st, cfg.ch_tiles):
                        cht = min(cfg.ch_tiles, st - c0)
                        gt = gpool.tile([P, cfg.ch_tiles, P], bf16, tag="G")
                        nidx = cht * P
                        nc.gpsimd.dma_gather(
                            out_ap=gt[:, 0:cht, :],
                            in_ap=xg[qq * cfg.qsize:(qq + 1) * cfg.qsize, :],
                            idxs_ap=ixs[:, c0 * 8:(c0 + cht) * 8],
                            num_idxs=nidx, num_idxs_reg=nidx, elem_size=P,
                            single_packet=False)
                        for t in range(cht):
                            g = g_tile0 + c0 + t
                            s_t = spool.tile([P, P], bf16, tag="S")
                            nc.vector.tensor_scalar(
                                out=s_t[:], in0=iota_sb[:],
                                scalar1=dloc_sb[:, g:g + 1],
                                scalar2=ew_sb[:, g:g + 1],
                                op0=Alu.is_equal, op1=Alu.mult)
                            b = int(sched["t_bh"][g])
                            nc.tensor.matmul(
                                out=ps_hq[:, b * fdim:(b + 1) * fdim],
                                lhsT=s_t[:], rhs=gt[:, t, 0:fdim],
                                start=bool(sched["t_j"][g] == 0),
                                stop=bool(sched["t_j"][g]
                                          == sched["T"][sched["tile_run"][g]] - 1))
                    g_tile0 += st
                    nc.vector.tensor_tensor(out=acc[:], in0=acc[:],
                                            in1=ps_hq[:], op=Alu.add)
                epilogue(h, acc)

        # L1 epilogue: h1s = relu(dinv*agg + b1) * dinv
        h1pool = ctx.enter_context(tc.tile_pool(name="h1s", bufs=1))
        h1s_sb = h1pool.tile([P, nblk * f_hid], f32)

        def epi1(h, ps_half):
            with tc.tile_pool(name="epi1", bufs=4) as ep:
                for b in range(nbh):
                    gb = h * nbh + b
                    if dbg:
                        dc = ep.tile([P, f_hid], f32, tag="dbgc")
                        nc.vector.tensor_copy(
                            out=dc[:],
                            in_=ps_half[:, b * f_hid:(b + 1) * f_hid])
                        nc.sync.dma_start(
                            out=dbg_agg1[:, gb * f_hid:(gb + 1) * f_hid],
                            in_=dc[:])
                    u = ep.tile([P, f_hid], f32, tag="u")
                    nc.vector.tensor_scalar(
                        out=u[:], in0=ps_half[:, b * f_hid:(b + 1) * f_hid],
                        scalar1=dinv_sb[:, gb:gb + 1], scalar2=None,
                        op0=Alu.mult)
                    v = ep.tile([P, f_hid], f32, tag="v")
                    nc.vector.tensor_tensor(
                        out=v[:], in0=u[:], in1=b1b_sb[:], op=Alu.add)
                    nc.vector.tensor_scalar(
                        out=h1s_sb[:, gb * f_hid:(gb + 1) * f_hid], in0=v[:],
                        scalar1=0.0, scalar2=dinv_sb[:, gb:gb + 1],
                        op0=Alu.max, op1=Alu.mult)

        ctx_pools = {
            "g": ctx.enter_context(tc.tile_pool(name="gpool", bufs=3)),
            "s": ctx.enter_context(tc.tile_pool(name="spool", bufs=6)),
        }

        accpool = ctx.enter_context(tc.tile_pool(name="aggacc", bufs=1))
        with tc.tile_pool(name="aggps1", bufs=1, space="PSUM") as pspool1:
            emit_agg(xg1, f_hid, epi1, pspool1, accpool)
        if dbg:
            nc.sync.dma_start(out=dbg_h1s[:], in_=h1s_sb[:])

        # ---- xt2 = h1s @ W2 (h1s already carries the dinv source scale)
        with tc.tile_pool(name="x2prep", bufs=3) as xp2, \
                tc.tile_pool(name="x2ps", bufs=2, space="PSUM") as xps2:
            for gb in range(nblk):
                w = min(P, ns - gb * P)
                if w <= 0:
                    break
                ps_hT = xps2.tile([f_hid, P], f32, tag="ps_hT")
                nc.tensor.transpose(
                    out=ps_hT[:, :w],
                    in_=h1s_sb[:w, gb * f_hid:(gb + 1) * f_hid],
                    identity=ident_sb[:w, :w])
                hT = xp2.tile([f_hid, P], f32, tag="hT")
                nc.vector.tensor_copy(out=hT[:, :w], in_=ps_hT[:, :w])
                ps_x2 = xps2.tile([f_out, P], f32, tag="ps_x2")
                nc.tensor.matmul(out=ps_x2[:, :w], lhsT=w2_sb[:],
                                 rhs=hT[:, :w], start=True, stop=True)
                x2T = xp2.tile([f_out, P], f32, tag="x2T")
                nc.vector.tensor_copy(out=x2T[:, :w], in_=ps_x2[:, :w])
                ps_t2 = xps2.tile([P, f_out], f32, tag="ps_t2")
                nc.tensor.transpose(out=ps_t2[:w, :], in_=x2T[:, :w],
                                    identity=ident_sb[:f_out, :f_out])
                x2b = xp2.tile([P, f_out], bf16, tag="x2b")
                nc.vector.tensor_copy(out=x2b[:w, :], in_=ps_t2[:w, :])
                nc.sync.dma_start(out=xloc2[gb * P:gb * P + w, 0:f_out],
                                  in_=x2b[:w, :])

        nc.gpsimd.collective_compute(
            "AllGather", Alu.bypass, replica_groups=groups,
            ins=[xloc2[:]], outs=[xg2[:]])
        if dbg:
            nc.sync.dma_start(out=dbg_xg2[:], in_=xg2[:])

        # L2 epilogue: out = dinv*agg + b2 -> DRAM
        def epi2(h, ps_half):
            with tc.tile_pool(name="epi2", bufs=4) as ep:
                for b in range(nbh):
                    gb = h * nbh + b
                    u = ep.tile([P, f_out], f32, tag="u2")
                    nc.vector.tensor_scalar(
                        out=u[:], in0=ps_half[:, b * f_out:(b + 1) * f_out],
                        scalar1=dinv_sb[:, gb:gb + 1], scalar2=None,
                        op0=Alu.mult)
                    o = ep.tile([P, f_out], f32, tag="o2")
                    nc.vector.tensor_tensor(
                        out=o[:], in0=u[:], in1=b2b_sb[:], op=Alu.add)
                    nc.sync.dma_start(out=out_d[gb * P:(gb + 1) * P, :],
                                      in_=o[:])

        with tc.tile_pool(name="aggps2", bufs=1, space="PSUM") as pspool2:
            emit_agg(xg2, f_out, epi2, pspool2, accpool)

    nc.compile()
    return nc


# ---------------------------------------------------------------- entry point

def _run(cfg: Cfg, z, edge_index, edge_attr, W1, b1, W2, b2, dbg=False):
    import ml_dtypes
    from concourse.bass_utils import run_bass_kernel_spmd

    import time as _time
    _t = _time.time()
    sched, per_core = preprocess(cfg, np.asarray(edge_index),
                                 np.asarray(edge_attr, dtype=np.float32))
    print(f"[kernel] preprocess {_time.time()-_t:.1f}s "
          f"tiles/layer={sched['total_tiles']}", flush=True)
    _t = _time.time()
    nc = build_program(cfg, sched, dbg=dbg)
    print(f"[kernel] build+schedule {_time.time()-_t:.1f}s", flush=True)

    z = np.asarray(z, dtype=np.float32)
    W1 = np.asarray(W1, dtype=np.float32)
    b1 = np.asarray(b1, dtype=np.float32)
    W2 = np.asarray(W2, dtype=np.float32)
    b2 = np.asarray(b2, dtype=np.float32)

    iota = np.tile(
        np.arange(P, dtype=np.float32).astype(ml_dtypes.bfloat16)[None, :],
        (P, 1))
    ident = np.eye(P, dtype=np.float32)
    b1b = np.tile(b1[None, :], (P, 1)).astype(np.float32)
    b2b = np.tile(b2[None, :], (P, 1)).astype(np.float32)

    in_maps = []
    for c in range(cfg.ncores):
        pc = per_core[c]
        zt = np.ascontiguousarray(
            z[c * cfg.nshard:(c + 1) * cfg.nshard, :].T)
        m = {
            "zT": zt, "W1": W1, "W2": W2, "b1b": b1b, "b2b": b2b,
            "iota": iota, "ident": ident, "degpad": pc["degpad"],
            "dloc": pc["dloc_col"], "ew": pc["ew_col"],
        }
        for h in range(2):
            for qq in range(cfg.qn):
                m[f"idx_h{h}q{qq}"] = pc["idx_segs"][(h, qq)]
        in_maps.append(m)

    _t = _time.time()
    res = run_bass_kernel_spmd(
        nc, in_maps, core_ids=list(range(cfg.ncores)),
        trace=bool(int(__import__("os").environ.get("KERNEL_TRACE", "0"))))
    print(f"[kernel] compile+run {_time.time()-_t:.1f}s", flush=True)

    out = np.concatenate(
        [res.results[c]["out"][:cfg.nshard] for c in range(cfg.ncores)], axis=0)
    return out.astype(np.float32), res


def kernel(z, edge_index, edge_attr, W1, b1, W2, b2):
    out, _ = _run(FULL_CFG, z, edge_index, edge_attr, W1, b1, W2, b2)
    return out



# revision 4
# speedup vs baseline: 1.2077x; 1.2077x over previous
"""GCN 2-layer decoder on 8 trn2 NeuronCores.

Algorithm (per core, nodes dest-sharded):
  deg[c]  = sum of in-edge weights (+1 self loop)   [host pads slots, DVE reduce]
  dinv    = 1/sqrt(deg)
  xt1[r]  = dinv[r] * (z @ W1)[r]      -> bf16 rows in a Shared DRAM table
  agg[c]  = sum_e ew_e * xt1[row_e]    [dma_gather rows + selector-matmul in PSUM]
  h1s[c]  = relu(dinv[c]*agg[c] + b1) * dinv[c]
  xt2[r]  = (h1s @ W2)[r]              -> bf16 rows in Shared table
  out[c]  = dinv[c] * (sum_e ew_e * xt2[row_e]) + b2

Edges are sorted by (dest-half, source-quarter, dest-block); each
(half, quarter, block) run is padded to a uniform (cross-core) tile count so
the single SPMD program works for all 8 cores.  Source rows are fetched with
gpsimd.dma_gather (int16 quarter-local indices); per 128-edge tile a [128,128]
bf16 selector S (S[e,d] = ew_e * (d == dloc_e%128)) is built with one DVE
tensor_scalar and PE accumulates S.T @ G into the block's PSUM column.
"""

import math
from contextlib import ExitStack
from dataclasses import dataclass

import numpy as np

P = 128


@dataclass(frozen=True)
class Cfg:
    n: int              # total nodes
    ncores: int         # 8
    qn: int             # source quarters (index range per gather table slice)
    f_in: int           # 64
    f_hid: int          # 64
    f_out: int          # 32
    ch_tiles: int = 32  # gather chunk size in 128-edge tiles

    @property
    def nshard(self):
        return self.n // self.ncores

    @property
    def nblk(self):
        return math.ceil(self.nshard / P)

    @property
    def nblk_h(self):
        return math.ceil(self.nblk / 2)

    @property
    def dests_pad(self):
        return self.nblk * P

    @property
    def qsize(self):
        return self.n // self.qn


FULL_CFG = Cfg(n=100000, ncores=8, qn=4, f_in=64, f_hid=64, f_out=32)


# ---------------------------------------------------------------- host side

def preprocess(cfg: Cfg, edge_index: np.ndarray, edge_attr: np.ndarray):
    """Build the uniform schedule + per-core device input arrays."""
    n = cfg.n
    ns = cfg.nshard
    nbh = cfg.nblk_h

    rows = np.concatenate([edge_index[0], np.arange(n, dtype=np.int64)])
    cols = np.concatenate([edge_index[1], np.arange(n, dtype=np.int64)])
    ews = np.concatenate([edge_attr.astype(np.float32),
                          np.ones(n, dtype=np.float32)])

    core = cols // ns
    dloc = (cols - core * ns).astype(np.int64)
    q = rows // cfg.qsize
    rloc = (rows - q * cfg.qsize).astype(np.int64)
    blk = dloc // P
    half = (blk >= nbh).astype(np.int64)
    bh = blk - half * nbh  # block within half

    assert rloc.max() < 32768, "quarter-local index must fit int16"

    # run id in schedule order: (half, quarter, block-in-half)
    run_id = (half * cfg.qn + q) * nbh + bh
    n_runs = 2 * cfg.qn * nbh

    # counts per (core, run)
    cnt = np.zeros((cfg.ncores, n_runs), dtype=np.int64)
    np.add.at(cnt, (core, run_id), 1)
    T = np.maximum(1, np.ceil(cnt.max(axis=0) / P).astype(np.int64))  # [n_runs]

    run_tile_off = np.concatenate([[0], np.cumsum(T)])   # tile offset per run
    total_tiles = int(run_tile_off[-1])                   # tiles per layer
    total_slots = total_tiles * P

    # per-run tile metadata (uniform across cores)
    tile_run = np.repeat(np.arange(n_runs), T)            # [total_tiles]
    t_half = tile_run // (cfg.qn * nbh)
    t_q = (tile_run // nbh) % cfg.qn
    t_bh = tile_run % nbh
    # j = tile index within run
    t_j = np.arange(total_tiles) - run_tile_off[tile_run]
    t_start = (t_q == 0) & (t_j == 0)
    last_j = T[tile_run] - 1
    t_stop = (t_q == cfg.qn - 1) & (t_j == last_j)

    # per-(half,q) segment boundaries in tile units
    seg_tiles = {}
    for h in range(2):
        for qq in range(cfg.qn):
            r0 = (h * cfg.qn + qq) * nbh
            seg_tiles[(h, qq)] = int(T[r0:r0 + nbh].sum())

    sched = {
        "T": T, "tile_run": tile_run, "t_half": t_half, "t_q": t_q,
        "t_bh": t_bh, "t_start": t_start, "t_stop": t_stop, "t_j": t_j,
        "run_tile_off": run_tile_off, "total_tiles": total_tiles,
        "seg_tiles": seg_tiles,
    }

    # degree slot count (uniform): max in-degree over all nodes
    deg_cnt = np.bincount(cols, minlength=n)  # includes self loops
    dslot = int(math.ceil((deg_cnt.max() + 1) / 8) * 8)
    sched["dslot"] = dslot

    per_core = []
    order_all = np.lexsort((dloc, run_id, core))  # sorted by core, run, dloc
    core_sorted = core[order_all]
    core_bounds = np.searchsorted(core_sorted, np.arange(cfg.ncores + 1))

    for c in range(cfg.ncores):
        sel = order_all[core_bounds[c]:core_bounds[c + 1]]
        c_run = run_id[sel]
        c_rloc = rloc[sel]
        c_dloc = dloc[sel]
        c_ew = ews[sel]

        # rank within run (sel is sorted by run)
        run_starts = np.searchsorted(c_run, np.arange(n_runs))
        rank = np.arange(len(sel)) - run_starts[c_run]
        slot = (run_tile_off[c_run] * P + rank).astype(np.int64)

        s_rloc = np.zeros(total_slots, dtype=np.int16)
        s_dlocrel = np.zeros(total_slots, dtype=np.float32)
        s_ew = np.zeros(total_slots, dtype=np.float32)
        s_rloc[slot] = c_rloc.astype(np.int16)
        s_dlocrel[slot] = (c_dloc % P).astype(np.float32)
        s_ew[slot] = c_ew

        # dloc/ew in [128, total_tiles] device layout
        dloc_col = np.ascontiguousarray(s_dlocrel.reshape(total_tiles, P).T)
        ew_col = np.ascontiguousarray(s_ew.reshape(total_tiles, P).T)

        # idx arrays per (h,q) segment, wrapped 16 + replicated to 128 parts
        idx_segs = {}
        t0 = 0
        for h in range(2):
            for qq in range(cfg.qn):
                st = seg_tiles[(h, qq)]
                seg = s_rloc[t0 * P:(t0 + st) * P]
                wrapped = np.ascontiguousarray(seg.reshape(-1, 16).T)  # [16, S/16]
                idx_segs[(h, qq)] = np.ascontiguousarray(
                    np.tile(wrapped, (P // 16, 1)))
                t0 += st

        # degree pad array [128, nblk*dslot]
        dmask = (cols // ns) == c
        dd = dloc[dmask]
        dw = ews[dmask]
        o2 = np.argsort(dd, kind="stable")
        dd, dw = dd[o2], dw[o2]
        dstart = np.searchsorted(dd, np.arange(ns))
        drank = np.arange(len(dd)) - dstart[dd]
        degpad = np.zeros((P, cfg.nblk * dslot), dtype=np.float32)
        degpad[dd % P, (dd // P) * dslot + drank] = dw
        # phantom dests get deg=1 to avoid 1/0
        for ph in range(ns, cfg.dests_pad):
            degpad[ph % P, (ph // P) * dslot] = 1.0

        per_core.append({
            "dloc_col": dloc_col, "ew_col": ew_col, "idx_segs": idx_segs,
            "degpad": degpad,
        })

    return sched, per_core


# ---------------------------------------------------------------- device side

def build_program(cfg: Cfg, sched, dbg: bool = False):
    import ml_dtypes  # noqa: F401
    from concourse import bacc, bass, mybir, tile
    from concourse.library_config import mlp

    f32 = mybir.dt.float32
    bf16 = mybir.dt.bfloat16
    i16 = mybir.dt.int16
    Alu = mybir.AluOpType
    Act = mybir.ActivationFunctionType

    n, ns, nbh, nblk = cfg.n, cfg.nshard, cfg.nblk_h, cfg.nblk
    dslot = sched["dslot"]
    TT = sched["total_tiles"]
    f_in, f_hid, f_out = cfg.f_in, cfg.f_hid, cfg.f_out

    nc = bacc.Bacc("TRN2", target_bir_lowering=False, debug=False,
                   enable_asserts=False, num_devices=cfg.ncores,
                   num_swdge_queues=4)

    # ---- I/O declarations
    zT_d = nc.dram_tensor("zT", [f_in, ns], f32, kind="ExternalInput")
    w1_d = nc.dram_tensor("W1", [f_in, f_hid], f32, kind="ExternalInput")
    w2_d = nc.dram_tensor("W2", [f_hid, f_out], f32, kind="ExternalInput")
    b1b_d = nc.dram_tensor("b1b", [P, f_hid], f32, kind="ExternalInput")
    b2b_d = nc.dram_tensor("b2b", [P, f_out], f32, kind="ExternalInput")
    iota_d = nc.dram_tensor("iota", [P, P], bf16, kind="ExternalInput")
    ident_d = nc.dram_tensor("ident", [P, P], f32, kind="ExternalInput")
    degpad_d = nc.dram_tensor("degpad", [P, nblk * dslot], f32,
                              kind="ExternalInput")
    dloc_d = nc.dram_tensor("dloc", [P, TT], f32, kind="ExternalInput")
    ew_d = nc.dram_tensor("ew", [P, TT], f32, kind="ExternalInput")
    idx_d = {}
    for h in range(2):
        for qq in range(cfg.qn):
            st = sched["seg_tiles"][(h, qq)]
            idx_d[(h, qq)] = nc.dram_tensor(
                f"idx_h{h}q{qq}", [P, st * P // 16], i16, kind="ExternalInput")
    out_d = nc.dram_tensor("out", [cfg.dests_pad, f_out], f32,
                           kind="ExternalOutput")
    if dbg:
        dbg_dinv = nc.dram_tensor("dbg_dinv", [P, nblk], f32,
                                  kind="ExternalOutput")
        dbg_xg1 = nc.dram_tensor("dbg_xg1", [n, P], bf16,
                                 kind="ExternalOutput")
        dbg_agg1 = nc.dram_tensor("dbg_agg1", [P, nblk * f_hid], f32,
                                  kind="ExternalOutput")
        dbg_h1s = nc.dram_tensor("dbg_h1s", [P, nblk * f_hid], f32,
                                 kind="ExternalOutput")
        dbg_xg2 = nc.dram_tensor("dbg_xg2", [n, P], bf16,
                                 kind="ExternalOutput")

    # local slice + shared gathered tables (rows padded to 128 bf16 = 256B)
    xloc1 = nc.dram_tensor("xloc1", [ns, P], bf16, kind="Internal")
    xg1 = nc.dram_tensor("xg1", [n, P], bf16, kind="Internal",
                         addr_space="Shared")
    xloc2 = nc.dram_tensor("xloc2", [ns, P], bf16, kind="Internal")
    xg2 = nc.dram_tensor("xg2", [n, P], bf16, kind="Internal",
                         addr_space="Shared")

    groups = [list(range(cfg.ncores))]

    with tile.TileContext(nc, num_cores=cfg.ncores) as tc, \
            ExitStack() as ctx:
        nc.gpsimd.load_library(mlp)

        cpool = ctx.enter_context(tc.tile_pool(name="const", bufs=1))

        def load_const(dram, shape, dtype, tag):
            t = cpool.tile(shape, dtype, tag=tag)
            nc.sync.dma_start(out=t[:], in_=dram[:])
            return t

        iota_sb = load_const(iota_d, [P, P], bf16, "iota")
        ident_sb = load_const(ident_d, [P, P], f32, "ident")
        b1b_sb = load_const(b1b_d, [P, f_hid], f32, "b1b")
        b2b_sb = load_const(b2b_d, [P, f_out], f32, "b2b")
        w1_sb = load_const(w1_d, [f_in, f_hid], f32, "w1")
        w2_sb = load_const(w2_d, [f_hid, f_out], f32, "w2")
        dloc_sb = load_const(dloc_d, [P, TT], f32, "dloc")
        ew_sb = load_const(ew_d, [P, TT], f32, "ew")
        idx_sb = {}
        for h in range(2):
            for qq in range(cfg.qn):
                st = sched["seg_tiles"][(h, qq)]
                idx_sb[(h, qq)] = load_const(idx_d[(h, qq)],
                                             [P, st * P // 16], i16,
                                             f"idx{h}{qq}")

        # ---- deg -> dinv
        dinv_sb = cpool.tile([P, nblk], f32, tag="dinv")
        with tc.tile_pool(name="deg", bufs=1) as dpool:
            degpad_sb = dpool.tile([P, nblk * dslot], f32)
            nc.sync.dma_start(out=degpad_sb[:], in_=degpad_d[:])
            deg_sb = dpool.tile([P, nblk], f32)
            nc.vector.tensor_reduce(
                out=deg_sb[:],
                in_=degpad_sb[:].rearrange("p (b s) -> p b s", s=dslot),
                axis=mybir.AxisListType.X, op=Alu.add)
            rdeg_sb = dpool.tile([P, nblk], f32)
            nc.vector.reciprocal(out=rdeg_sb[:], in_=deg_sb[:])
            nc.scalar.activation(out=dinv_sb[:], in_=rdeg_sb[:], func=Act.Sqrt)
            if dbg:
                nc.sync.dma_start(out=dbg_dinv[:], in_=dinv_sb[:])

        # ---- xt1 = dinv * (z @ W1), written as bf16 rows of xloc1
        def emit_xt_prep(src_get, w_sb, fdim_in, fdim_out, xloc, scale):
            """src_get(chunk)->AP [fdim_in, width] feature-major source."""
            with tc.tile_pool(name="xprep", bufs=3) as xp, \
                    tc.tile_pool(name="xprep_ps", bufs=3, space="PSUM") as xps:
                nchunks = math.ceil(ns / 512)
                for ch in range(nchunks):
                    n0 = ch * 512
                    width = min(512, ns - n0)
                    ps_x = xps.tile([fdim_out, 512], f32, tag="ps_x")
                    nc.tensor.matmul(out=ps_x[:, :width], lhsT=w_sb[:],
                                     rhs=src_get(ch, width), start=True,
                                     stop=True)
                    xT = xp.tile([fdim_out, 512], f32, tag="xT")
                    nc.vector.tensor_copy(out=xT[:, :width], in_=ps_x[:, :width])
                    for j in range(math.ceil(width / P)):
                        nb = ch * 4 + j
                        w = min(P, width - j * P)
                        ps_t = xps.tile([P, fdim_out], f32, tag="ps_t")
                        nc.tensor.transpose(
                            out=ps_t[:w, :], in_=xT[:, j * P:j * P + w],
                            identity=ident_sb[:fdim_out, :fdim_out])
                        xb = xp.tile([P, fdim_out], bf16, tag="xb")
                        if scale:
                            nc.vector.tensor_scalar(
                                out=xb[:w, :], in0=ps_t[:w, :],
                                scalar1=dinv_sb[:w, nb:nb + 1], scalar2=None,
                                op0=Alu.mult)
                        else:
                            nc.vector.tensor_copy(out=xb[:w, :], in_=ps_t[:w, :])
                        nc.sync.dma_start(
                            out=xloc[n0 + j * P:n0 + j * P + w, 0:fdim_out],
                            in_=xb[:w, :])

        with tc.tile_pool(name="zt", bufs=1) as zpool:
            zT_sb = zpool.tile([f_in, ns], f32)
            nc.sync.dma_start(out=zT_sb[:], in_=zT_d[:])
            emit_xt_prep(lambda ch, w: zT_sb[:, ch * 512:ch * 512 + w],
                         w1_sb, f_in, f_hid, xloc1, scale=True)

        nc.gpsimd.collective_compute(
            "AllGather", Alu.bypass, replica_groups=groups,
            ins=[xloc1[:]], outs=[xg1[:]])
        if dbg:
            nc.sync.dma_start(out=dbg_xg1[:], in_=xg1[:])

        # ---- aggregation layer
        def emit_agg(xg, fdim, epilogue, pspool, accpool):
            gpool = ctx_pools["g"]
            spool = ctx_pools["s"]
            g_tile0 = 0
            chunk_no = 0
            for h in range(2):
                acc = accpool.tile([P, nbh * fdim], f32, tag="agg_acc")
                nc.vector.memset(acc[:], 0.0)
                for qq in range(cfg.qn):
                    ps_hq = pspool.tile([P, nbh * fdim], f32, tag="ps_hq")
                    st = sched["seg_tiles"][(h, qq)]
                    ixs = idx_sb[(h, qq)]
                    for c0 in range(0, st, cfg.ch_tiles):
                        cht = min(cfg.ch_tiles, st - c0)
                        gt = gpool.tile([P, cfg.ch_tiles, P], bf16, tag="G")
                        nidx = cht * P
                        nc.gpsimd.dma_gather(
                            out_ap=gt[:, 0:cht, :],
                            in_ap=xg[qq * cfg.qsize:(qq + 1) * cfg.qsize, :],
                            idxs_ap=ixs[:, c0 * 8:(c0 + cht) * 8],
                            num_idxs=nidx, num_idxs_reg=nidx, elem_size=P,
                            single_packet=False,
                            queue_num=chunk_no % 4)
                        chunk_no += 1
                        for t in range(cht):
                            g = g_tile0 + c0 + t
                            s_t = spool.tile([P, P], bf16, tag="S")
                            nc.vector.tensor_scalar(
                                out=s_t[:], in0=iota_sb[:],
                                scalar1=dloc_sb[:, g:g + 1],
                                scalar2=ew_sb[:, g:g + 1],
                                op0=Alu.is_equal, op1=Alu.mult)
                            b = int(sched["t_bh"][g])
                            nc.tensor.matmul(
                                out=ps_hq[:, b * fdim:(b + 1) * fdim],
                                lhsT=s_t[:], rhs=gt[:, t, 0:fdim],
                                start=bool(sched["t_j"][g] == 0),
                                stop=bool(sched["t_j"][g]
                                          == sched["T"][sched["tile_run"][g]] - 1))
                    g_tile0 += st
                    nc.vector.tensor_tensor(out=acc[:], in0=acc[:],
                                            in1=ps_hq[:], op=Alu.add)
                epilogue(h, acc)

        # L1 epilogue: h1s = relu(dinv*agg + b1) * dinv
        h1pool = ctx.enter_context(tc.tile_pool(name="h1s", bufs=1))
        h1s_sb = h1pool.tile([P, nblk * f_hid], f32)

        def epi1(h, ps_half):
            with tc.tile_pool(name="epi1", bufs=4) as ep:
                for b in range(nbh):
                    gb = h * nbh + b
                    if dbg:
                        dc = ep.tile([P, f_hid], f32, tag="dbgc")
                        nc.vector.tensor_copy(
                            out=dc[:],
                            in_=ps_half[:, b * f_hid:(b + 1) * f_hid])
                        nc.sync.dma_start(
                            out=dbg_agg1[:, gb * f_hid:(gb + 1) * f_hid],
                            in_=dc[:])
                    u = ep.tile([P, f_hid], f32, tag="u")
                    nc.vector.tensor_scalar(
                        out=u[:], in0=ps_half[:, b * f_hid:(b + 1) * f_hid],
                        scalar1=dinv_sb[:, gb:gb + 1], scalar2=None,
                        op0=Alu.mult)
                    v = ep.tile([P, f_hid], f32, tag="v")
                    nc.vector.tensor_tensor(
                        out=v[:], in0=u[:], in1=b1b_sb[:], op=Alu.add)
                    nc.vector.tensor_scalar(
                        out=h1s_sb[:, gb * f_hid:(gb + 1) * f_hid], in0=v[:],
                        scalar1=0.0, scalar2=dinv_sb[:, gb:gb + 1],
                        op0=Alu.max, op1=Alu.mult)

        ctx_pools = {
            "g": ctx.enter_context(tc.tile_pool(name="gpool", bufs=6)),
            "s": ctx.enter_context(tc.tile_pool(name="spool", bufs=6)),
        }

        accpool = ctx.enter_context(tc.tile_pool(name="aggacc", bufs=1))
        with tc.tile_pool(name="aggps1", bufs=1, space="PSUM") as pspool1:
            emit_agg(xg1, f_hid, epi1, pspool1, accpool)
        if dbg:
            nc.sync.dma_start(out=dbg_h1s[:], in_=h1s_sb[:])

        # ---- xt2 = h1s @ W2 (h1s already carries the dinv source scale)
        with tc.tile_pool(name="x2prep", bufs=3) as xp2, \
                tc.tile_pool(name="x2ps", bufs=2, space="PSUM") as xps2:
            for gb in range(nblk):
                w = min(P, ns - gb * P)
                if w <= 0:
                    break
                ps_hT = xps2.tile([f_hid, P], f32, tag="ps_hT")
                nc.tensor.transpose(
                    out=ps_hT[:, :w],
                    in_=h1s_sb[:w, gb * f_hid:(gb + 1) * f_hid],
                    identity=ident_sb[:w, :w])
                hT = xp2.tile([f_hid, P], f32, tag="hT")
                nc.vector.tensor_copy(out=hT[:, :w], in_=ps_hT[:, :w])
                ps_x2 = xps2.tile([f_out, P], f32, tag="ps_x2")
                nc.tensor.matmul(out=ps_x2[:, :w], lhsT=w2_sb[:],
                                 rhs=hT[:, :w], start=True, stop=True)
                x2T = xp2.tile([f_out, P], f32, tag="x2T")
                nc.vector.tensor_copy(out=x2T[:, :w], in_=ps_x2[:, :w])
                ps_t2 = xps2.tile([P, f_out], f32, tag="ps_t2")
                nc.tensor.transpose(out=ps_t2[:w, :], in_=x2T[:, :w],
                                    identity=ident_sb[:f_out, :f_out])
                x2b = xp2.tile([P, f_out], bf16, tag="x2b")
                nc.vector.tensor_copy(out=x2b[:w, :], in_=ps_t2[:w, :])
                nc.sync.dma_start(out=xloc2[gb * P:gb * P + w, 0:f_out],
                                  in_=x2b[:w, :])

        nc.gpsimd.collective_compute(
            "AllGather", Alu.bypass, replica_groups=groups,
            ins=[xloc2[:]], outs=[xg2[:]])
        if dbg:
            nc.sync.dma_start(out=dbg_xg2[:], in_=xg2[:])

        # L2 epilogue: out = dinv*agg + b2 -> DRAM
        def epi2(h, ps_half):
            with tc.tile_pool(name="epi2", bufs=4) as ep:
                for b in range(nbh):
                    gb = h * nbh + b
                    u = ep.tile([P, f_out], f32, tag="u2")
                    nc.vector.tensor_scalar(
                        out=u[:], in0=ps_half[:, b * f_out:(b + 1) * f_out],
                        scalar1=dinv_sb[:, gb:gb + 1], scalar2=None,
                        op0=Alu.mult)
                    o = ep.tile([P, f_out], f32, tag="o2")
                    nc.vector.tensor_tensor(
                        out=o[:], in0=u[:], in1=b2b_sb[:], op=Alu.add)
                    nc.sync.dma_start(out=out_d[gb * P:(gb + 1) * P, :],
                                      in_=o[:])

        with tc.tile_pool(name="aggps2", bufs=1, space="PSUM") as pspool2:
            emit_agg(xg2, f_out, epi2, pspool2, accpool)

    nc.compile()
    return nc


# ---------------------------------------------------------------- entry point

def _run(cfg: Cfg, z, edge_index, edge_attr, W1, b1, W2, b2, dbg=False):
    import ml_dtypes
    from concourse.bass_utils import run_bass_kernel_spmd

    import time as _time
    _t = _time.time()
    sched, per_core = preprocess(cfg, np.asarray(edge_index),
                                 np.asarray(edge_attr, dtype=np.float32))
    print(f"[kernel] preprocess {_time.time()-_t:.1f}s "
          f"tiles/layer={sched['total_tiles']}", flush=True)
    _t = _time.time()
    nc = build_program(cfg, sched, dbg=dbg)
    print(f"[kernel] build+schedule {_time.time()-_t:.1f}s", flush=True)

    z = np.asarray(z, dtype=np.float32)
    W1 = np.asarray(W1, dtype=np.float32)
    b1 = np.asarray(b1, dtype=np.float32)
    W2 = np.asarray(W2, dtype=np.float32)
    b2 = np.asarray(b2, dtype=np.float32)

    iota = np.tile(
        np.arange(P, dtype=np.float32).astype(ml_dtypes.bfloat16)[None, :],
        (P, 1))
    ident = np.eye(P, dtype=np.float32)
    b1b = np.tile(b1[None, :], (P, 1)).astype(np.float32)
    b2b = np.tile(b2[None, :], (P, 1)).astype(np.float32)

    in_maps = []
    for c in range(cfg.ncores):
        pc = per_core[c]
        zt = np.ascontiguousarray(
            z[c * cfg.nshard:(c + 1) * cfg.nshard, :].T)
        m = {
            "zT": zt, "W1": W1, "W2": W2, "b1b": b1b, "b2b": b2b,
            "iota": iota, "ident": ident, "degpad": pc["degpad"],
            "dloc": pc["dloc_col"], "ew": pc["ew_col"],
        }
        for h in range(2):
            for qq in range(cfg.qn):
                m[f"idx_h{h}q{qq}"] = pc["idx_segs"][(h, qq)]
        in_maps.append(m)

    _t = _time.time()
    res = run_bass_kernel_spmd(
        nc, in_maps, core_ids=list(range(cfg.ncores)),
        trace=bool(int(__import__("os").environ.get("KERNEL_TRACE", "0"))))
    print(f"[kernel] compile+run {_time.time()-_t:.1f}s", flush=True)

    out = np.concatenate(
        [res.results[c]["out"][:cfg.nshard] for c in range(cfg.ncores)], axis=0)
    return out.astype(np.float32), res


def kernel(z, edge_index, edge_attr, W1, b1, W2, b2):
    out, _ = _run(FULL_CFG, z, edge_index, edge_attr, W1, b1, W2, b2)
    return out



# revision 11
# speedup vs baseline: 1.8719x; 1.5500x over previous
"""GCN 2-layer decoder on 8 trn2 NeuronCores.

Algorithm (per core, nodes dest-sharded):
  deg[c]  = sum of in-edge weights (+1 self loop)   [host pads slots, DVE reduce]
  dinv    = 1/sqrt(deg)
  xt1[r]  = dinv[r] * (z @ W1)[r]      -> bf16 rows in a Shared DRAM table
  agg[c]  = xt1[c] + sum_e ew_e * xt1[row_e]
            [self-loop via identity-matmul PSUM init; edges via dma_gather
             rows + selector-matmul accumulated per half directly in PSUM]
  h1s[c]  = relu(dinv[c]*agg[c] + b1) * dinv[c]
  xt2[r]  = (h1s @ W2)[r]              -> bf16 rows in Shared table
  out[c]  = dinv[c] * (xt2[c] + sum_e ew_e * xt2[row_e]) + b2

Self-loop edges are excluded from the gathered edge list (their contribution
is the ident-matmul PSUM init).  Remaining edges are sorted by (dest-half,
source-quarter, dest-block); each (half, quarter, block) run is padded to a
uniform (cross-core) tile count so the single SPMD program works on all 8
cores.  Source rows are fetched with gpsimd.dma_gather round-robined over 4
SWDGE queues (each queue's descriptor generation runs on its own Q7 cpu
pair, so up to 4 gathers generate descriptors concurrently); per 128-edge
tile a [128,128] bf16 selector S (S[e,d] = ew_e * (d == dloc_e%128)) is
built 8 tiles at a time with two wide DVE tensor_tensor ops (stride-0
broadcast of the per-tile dloc/ew columns) and PE accumulates S.T @ G into
the block's PSUM column with start/stop spanning the whole half.
"""

import math
from contextlib import ExitStack
from dataclasses import dataclass

import numpy as np

P = 128
KB = 8  # tiles per batched selector build


@dataclass(frozen=True)
class Cfg:
    n: int              # total nodes
    ncores: int         # 8
    qn: int             # source quarters (index range per gather table slice)
    f_in: int           # 64
    f_hid: int          # 64
    f_out: int          # 32
    ch_tiles: int = 32  # gather chunk size in 128-edge tiles

    @property
    def nshard(self):
        return self.n // self.ncores

    @property
    def nblk(self):
        return math.ceil(self.nshard / P)

    @property
    def nblk_h(self):
        return math.ceil(self.nblk / 2)

    @property
    def dests_pad(self):
        return self.nblk * P

    @property
    def qsize(self):
        return self.n // self.qn


FULL_CFG = Cfg(n=100000, ncores=8, qn=4, f_in=64, f_hid=64, f_out=32)


# ---------------------------------------------------------------- host side

def preprocess(cfg: Cfg, edge_index: np.ndarray, edge_attr: np.ndarray):
    """Build the uniform schedule + per-core device input arrays."""
    import ml_dtypes

    n = cfg.n
    ns = cfg.nshard
    nbh = cfg.nblk_h

    # message edges exclude self loops (handled by ident-matmul PSUM init)
    rows = edge_index[0]
    cols = edge_index[1]
    ews = edge_attr.astype(np.float32)

    core = cols // ns
    dloc = (cols - core * ns).astype(np.int64)
    q = rows // cfg.qsize
    rloc = (rows - q * cfg.qsize).astype(np.int64)
    blk = dloc // P
    half = (blk >= nbh).astype(np.int64)
    bh = blk - half * nbh  # block within half

    assert rloc.max() < 32768, "quarter-local index must fit int16"

    # run id in schedule order: (half, quarter, block-in-half)
    run_id = (half * cfg.qn + q) * nbh + bh
    n_runs = 2 * cfg.qn * nbh

    # counts per (core, run)
    cnt = np.zeros((cfg.ncores, n_runs), dtype=np.int64)
    np.add.at(cnt, (core, run_id), 1)
    T = np.maximum(1, np.ceil(cnt.max(axis=0) / P).astype(np.int64))  # [n_runs]

    run_tile_off = np.concatenate([[0], np.cumsum(T)])   # tile offset per run
    total_tiles = int(run_tile_off[-1])                   # tiles per layer
    total_slots = total_tiles * P

    # per-run tile metadata (uniform across cores)
    tile_run = np.repeat(np.arange(n_runs), T)            # [total_tiles]
    t_q = (tile_run // nbh) % cfg.qn
    t_bh = tile_run % nbh
    t_j = np.arange(total_tiles) - run_tile_off[tile_run]
    last_j = T[tile_run] - 1
    t_stop = (t_q == cfg.qn - 1) & (t_j == last_j)

    # per-(half,q) segment boundaries in tile units
    seg_tiles = {}
    for h in range(2):
        for qq in range(cfg.qn):
            r0 = (h * cfg.qn + qq) * nbh
            seg_tiles[(h, qq)] = int(T[r0:r0 + nbh].sum())

    sched = {
        "T": T, "tile_run": tile_run, "t_bh": t_bh, "t_stop": t_stop,
        "run_tile_off": run_tile_off, "total_tiles": total_tiles,
        "seg_tiles": seg_tiles,
    }

    # degree includes self loops (weight 1)
    cols_deg = np.concatenate([cols, np.arange(n, dtype=np.int64)])
    ews_deg = np.concatenate([ews, np.ones(n, dtype=np.float32)])
    deg_cnt = np.bincount(cols_deg, minlength=n)
    dslot = int(math.ceil((deg_cnt.max() + 1) / 8) * 8)
    sched["dslot"] = dslot

    per_core = []
    order_all = np.lexsort((dloc, run_id, core))  # sorted by core, run, dloc
    core_sorted = core[order_all]
    core_bounds = np.searchsorted(core_sorted, np.arange(cfg.ncores + 1))

    for c in range(cfg.ncores):
        sel = order_all[core_bounds[c]:core_bounds[c + 1]]
        c_run = run_id[sel]
        c_rloc = rloc[sel]
        c_dloc = dloc[sel]
        c_ew = ews[sel]

        # rank within run (sel is sorted by run)
        run_starts = np.searchsorted(c_run, np.arange(n_runs))
        rank = np.arange(len(sel)) - run_starts[c_run]
        slot = (run_tile_off[c_run] * P + rank).astype(np.int64)

        s_rloc = np.zeros(total_slots, dtype=np.int16)
        s_dlocrel = np.zeros(total_slots, dtype=np.float32)
        s_ew = np.zeros(total_slots, dtype=np.float32)
        s_rloc[slot] = c_rloc.astype(np.int16)
        s_dlocrel[slot] = (c_dloc % P).astype(np.float32)
        s_ew[slot] = c_ew

        # dloc/ew in [128, total_tiles] device layout (bf16)
        dloc_col = np.ascontiguousarray(
            s_dlocrel.reshape(total_tiles, P).T).astype(ml_dtypes.bfloat16)
        ew_col = np.ascontiguousarray(
            s_ew.reshape(total_tiles, P).T).astype(ml_dtypes.bfloat16)

        # idx arrays per (h,q) segment, wrapped 16 + replicated to 128 parts
        idx_segs = {}
        t0 = 0
        for h in range(2):
            for qq in range(cfg.qn):
                st = seg_tiles[(h, qq)]
                seg = s_rloc[t0 * P:(t0 + st) * P]
                wrapped = np.ascontiguousarray(seg.reshape(-1, 16).T)
                idx_segs[(h, qq)] = np.ascontiguousarray(
                    np.tile(wrapped, (P // 16, 1)))
                t0 += st

        # degree pad array [128, nblk*dslot]
        dmask = (cols_deg // ns) == c
        dd = (cols_deg[dmask] - c * ns).astype(np.int64)
        dw = ews_deg[dmask]
        o2 = np.argsort(dd, kind="stable")
        dd, dw = dd[o2], dw[o2]
        dstart = np.searchsorted(dd, np.arange(ns))
        drank = np.arange(len(dd)) - dstart[dd]
        degpad = np.zeros((P, cfg.nblk * dslot), dtype=np.float32)
        degpad[dd % P, (dd // P) * dslot + drank] = dw
        # phantom dests get deg=1 to avoid 1/0
        for ph in range(ns, cfg.dests_pad):
            degpad[ph % P, (ph // P) * dslot] = 1.0

        per_core.append({
            "dloc_col": dloc_col, "ew_col": ew_col, "idx_segs": idx_segs,
            "degpad": degpad,
        })

    return sched, per_core


# ---------------------------------------------------------------- device side

def build_program(cfg: Cfg, sched, dbg: bool = False):
    import ml_dtypes  # noqa: F401
    from concourse import bacc, bass, mybir, tile
    from concourse.library_config import mlp

    f32 = mybir.dt.float32
    bf16 = mybir.dt.bfloat16
    i16 = mybir.dt.int16
    Alu = mybir.AluOpType
    Act = mybir.ActivationFunctionType

    n, ns, nbh, nblk = cfg.n, cfg.nshard, cfg.nblk_h, cfg.nblk
    dslot = sched["dslot"]
    TT = sched["total_tiles"]
    f_in, f_hid, f_out = cfg.f_in, cfg.f_hid, cfg.f_out

    nc = bacc.Bacc("TRN2", target_bir_lowering=False, debug=False,
                   enable_asserts=False, num_devices=cfg.ncores,
                   num_swdge_queues=4)

    # ---- I/O declarations
    zT_d = nc.dram_tensor("zT", [f_in, ns], f32, kind="ExternalInput")
    w1_d = nc.dram_tensor("W1", [f_in, f_hid], f32, kind="ExternalInput")
    w2_d = nc.dram_tensor("W2", [f_hid, f_out], f32, kind="ExternalInput")
    b1b_d = nc.dram_tensor("b1b", [P, f_hid], f32, kind="ExternalInput")
    b2b_d = nc.dram_tensor("b2b", [P, f_out], f32, kind="ExternalInput")
    iota_d = nc.dram_tensor("iota", [P, KB * P], bf16, kind="ExternalInput")
    ident_d = nc.dram_tensor("ident", [P, P], f32, kind="ExternalInput")
    identb_d = nc.dram_tensor("identb", [P, P], bf16, kind="ExternalInput")
    degpad_d = nc.dram_tensor("degpad", [P, nblk * dslot], f32,
                              kind="ExternalInput")
    dloc_d = nc.dram_tensor("dloc", [P, TT], bf16, kind="ExternalInput")
    ew_d = nc.dram_tensor("ew", [P, TT], bf16, kind="ExternalInput")
    idx_d = {}
    for h in range(2):
        for qq in range(cfg.qn):
            st = sched["seg_tiles"][(h, qq)]
            idx_d[(h, qq)] = nc.dram_tensor(
                f"idx_h{h}q{qq}", [P, st * P // 16], i16, kind="ExternalInput")
    out_d = nc.dram_tensor("out", [cfg.dests_pad, f_out], f32,
                           kind="ExternalOutput")
    if dbg:
        dbg_xg1 = nc.dram_tensor("dbg_xg1", [n, P], bf16,
                                 kind="ExternalOutput")
        dbg_h1s = nc.dram_tensor("dbg_h1s", [P, nblk * f_hid], f32,
                                 kind="ExternalOutput")
        dbg_dinv = nc.dram_tensor("dbg_dinv", [P, nblk], f32,
                                  kind="ExternalOutput")

    # local slice + shared gathered tables (rows padded to 128 bf16 = 256B)
    xloc1 = nc.dram_tensor("xloc1", [ns, P], bf16, kind="Internal")
    xg1 = nc.dram_tensor("xg1", [n, P], bf16, kind="Internal",
                         addr_space="Shared")
    xloc2 = nc.dram_tensor("xloc2", [ns, P], bf16, kind="Internal")
    xg2 = nc.dram_tensor("xg2", [n, P], bf16, kind="Internal",
                         addr_space="Shared")

    groups = [list(range(cfg.ncores))]

    with tile.TileContext(nc, num_cores=cfg.ncores) as tc, \
            ExitStack() as ctx:
        nc.gpsimd.load_library(mlp)

        cpool = ctx.enter_context(tc.tile_pool(name="const", bufs=1))

        def load_const(dram, shape, dtype, tag):
            t = cpool.tile(shape, dtype, tag=tag)
            nc.sync.dma_start(out=t[:], in_=dram[:])
            return t

        iota_sb = load_const(iota_d, [P, KB * P], bf16, "iota")
        ident_sb = load_const(ident_d, [P, P], f32, "ident")
        identb_sb = load_const(identb_d, [P, P], bf16, "identb")
        b1b_sb = load_const(b1b_d, [P, f_hid], f32, "b1b")
        b2b_sb = load_const(b2b_d, [P, f_out], f32, "b2b")
        w1_sb = load_const(w1_d, [f_in, f_hid], f32, "w1")
        w2_sb = load_const(w2_d, [f_hid, f_out], f32, "w2")
        dloc_sb = load_const(dloc_d, [P, TT], bf16, "dloc")
        ew_sb = load_const(ew_d, [P, TT], bf16, "ew")
        idx_sb = {}
        for h in range(2):
            for qq in range(cfg.qn):
                st = sched["seg_tiles"][(h, qq)]
                idx_sb[(h, qq)] = load_const(idx_d[(h, qq)],
                                             [P, st * P // 16], i16,
                                             f"idx{h}{qq}")

        # persistent bf16 copies of the shard's table rows (self-loop adds)
        xt1_sb = cpool.tile([P, nblk * f_hid], bf16, tag="xt1")
        xt2_sb = cpool.tile([P, nblk * f_out], bf16, tag="xt2")
        # last block has phantom rows the prep never writes; zero them
        nc.vector.memset(xt1_sb[:, (nblk - 1) * f_hid:nblk * f_hid], 0.0)
        nc.vector.memset(xt2_sb[:, (nblk - 1) * f_out:nblk * f_out], 0.0)

        # ---- deg -> dinv
        dinv_sb = cpool.tile([P, nblk], f32, tag="dinv")
        with tc.tile_pool(name="deg", bufs=1) as dpool:
            degpad_sb = dpool.tile([P, nblk * dslot], f32)
            nc.sync.dma_start(out=degpad_sb[:], in_=degpad_d[:])
            deg_sb = dpool.tile([P, nblk], f32)
            nc.vector.tensor_reduce(
                out=deg_sb[:],
                in_=degpad_sb[:].rearrange("p (b s) -> p b s", s=dslot),
                axis=mybir.AxisListType.X, op=Alu.add)
            rdeg_sb = dpool.tile([P, nblk], f32)
            nc.vector.reciprocal(out=rdeg_sb[:], in_=deg_sb[:])
            nc.scalar.activation(out=dinv_sb[:], in_=rdeg_sb[:], func=Act.Sqrt)

        # ---- xt1 = dinv * (z @ W1): write bf16 rows into xt1_sb + xloc1
        def emit_xt_prep(src_get, w_sb, fdim_in, fdim_out, xt_sb, xloc, scale):
            """src_get(chunk)->AP [fdim_in, width] feature-major source."""
            with tc.tile_pool(name="xprep", bufs=3) as xp, \
                    tc.tile_pool(name="xprep_ps", bufs=3, space="PSUM") as xps:
                nchunks = math.ceil(ns / 512)
                for ch in range(nchunks):
                    n0 = ch * 512
                    width = min(512, ns - n0)
                    ps_x = xps.tile([fdim_out, 512], f32, tag="ps_x")
                    nc.tensor.matmul(out=ps_x[:, :width], lhsT=w_sb[:],
                                     rhs=src_get(ch, width), start=True,
                                     stop=True)
                    xT = xp.tile([fdim_out, 512], f32, tag="xT")
                    nc.vector.tensor_copy(out=xT[:, :width], in_=ps_x[:, :width])
                    for j in range(math.ceil(width / P)):
                        nb = ch * 4 + j
                        w = min(P, width - j * P)
                        ps_t = xps.tile([P, fdim_out], f32, tag="ps_t")
                        nc.tensor.transpose(
                            out=ps_t[:w, :], in_=xT[:, j * P:j * P + w],
                            identity=ident_sb[:fdim_out, :fdim_out])
                        dst = xt_sb[:w, nb * fdim_out:(nb + 1) * fdim_out]
                        if scale:
                            nc.vector.tensor_scalar(
                                out=dst, in0=ps_t[:w, :],
                                scalar1=dinv_sb[:w, nb:nb + 1], scalar2=None,
                                op0=Alu.mult)
                        else:
                            nc.vector.tensor_copy(out=dst, in_=ps_t[:w, :])
                        nc.sync.dma_start(
                            out=xloc[n0 + j * P:n0 + j * P + w, 0:fdim_out],
                            in_=xt_sb[:w, nb * fdim_out:(nb + 1) * fdim_out])

        with tc.tile_pool(name="zt", bufs=1) as zpool:
            zT_sb = zpool.tile([f_in, ns], f32)
            nc.sync.dma_start(out=zT_sb[:], in_=zT_d[:])
            emit_xt_prep(lambda ch, w: zT_sb[:, ch * 512:ch * 512 + w],
                         w1_sb, f_in, f_hid, xt1_sb, xloc1, scale=True)

        nc.gpsimd.collective_compute(
            "AllGather", Alu.bypass, replica_groups=groups,
            ins=[xloc1[:]], outs=[xg1[:]])
        if dbg:
            nc.sync.dma_start(out=dbg_xg1[:], in_=xg1[:])
            nc.sync.dma_start(out=dbg_dinv[:], in_=dinv_sb[:])

        # ---- aggregation layer
        ctx_pools = {
            "g": ctx.enter_context(tc.tile_pool(name="gpool", bufs=10)),
            "s": ctx.enter_context(tc.tile_pool(name="spool", bufs=6)),
        }
        chunk_no = [0]

        def bcast_cols(tsb, g0, kb):
            ap = tsb[:, g0:g0 + kb]  # [P, kb]
            return bass.AP(ap.tensor, ap.offset,
                           [list(ap.ap[0]), list(ap.ap[1]), [0, P]])

        def emit_agg(xg, fdim, xt_sb, epilogue, pspool):
            gpool = ctx_pools["g"]
            spool = ctx_pools["s"]
            g_tile0 = 0
            for h in range(2):
                ps_h = pspool.tile([P, nbh * fdim], f32, tag="ps_h")
                # self-loop contribution initializes PSUM: ps[b] = xt[b].
                # One start=True matmul per 2KB PSUM bank (512 f32): a second
                # start=True in the same bank before a stop discards the
                # first session's contents.
                half_cols = nbh * fdim
                for col0 in range(0, half_cols, 512):
                    wcols = min(512, half_cols - col0)
                    nc.tensor.matmul(
                        out=ps_h[:, col0:col0 + wcols],
                        lhsT=identb_sb[:],
                        rhs=xt_sb[:, h * half_cols + col0:
                                  h * half_cols + col0 + wcols],
                        start=True, stop=False)
                for qq in range(cfg.qn):
                    st = sched["seg_tiles"][(h, qq)]
                    ixs = idx_sb[(h, qq)]
                    for c0 in range(0, st, cfg.ch_tiles):
                        cht = min(cfg.ch_tiles, st - c0)
                        gt = gpool.tile([P, cfg.ch_tiles, P], bf16, tag="G")
                        nidx = cht * P
                        nc.gpsimd.dma_gather(
                            out_ap=gt[:, 0:cht, :],
                            in_ap=xg[qq * cfg.qsize:(qq + 1) * cfg.qsize, :],
                            idxs_ap=ixs[:, c0 * 8:(c0 + cht) * 8],
                            num_idxs=nidx, num_idxs_reg=nidx, elem_size=P,
                            single_packet=False,
                            queue_num=chunk_no[0] % 4)
                        chunk_no[0] += 1
                        for b0 in range(0, cht, KB):
                            kb = min(KB, cht - b0)
                            g0 = g_tile0 + c0 + b0
                            s8 = spool.tile([P, KB * P], bf16, tag="S")
                            sv = s8[:, 0:kb * P].rearrange(
                                "p (k q) -> p k q", k=kb)
                            nc.vector.tensor_tensor(
                                out=sv,
                                in0=iota_sb[:, 0:kb * P].rearrange(
                                    "p (k q) -> p k q", k=kb),
                                in1=bcast_cols(dloc_sb, g0, kb),
                                op=Alu.is_equal)
                            nc.vector.tensor_tensor(
                                out=sv, in0=sv,
                                in1=bcast_cols(ew_sb, g0, kb),
                                op=Alu.mult)
                            for t in range(kb):
                                g = g0 + t
                                b = int(sched["t_bh"][g])
                                nc.tensor.matmul(
                                    out=ps_h[:, b * fdim:(b + 1) * fdim],
                                    lhsT=s8[:, t * P:(t + 1) * P],
                                    rhs=gt[:, b0 + t, 0:fdim],
                                    start=False,
                                    stop=bool(sched["t_stop"][g]))
                    g_tile0 += st
                epilogue(h, ps_h)

        # L1 epilogue: h1s = relu(dinv*ps + b1) * dinv
        h1pool = ctx.enter_context(tc.tile_pool(name="h1s", bufs=1))
        h1s_sb = h1pool.tile([P, nblk * f_hid], f32)

        def epi1(h, ps_h):
            with tc.tile_pool(name="epi1", bufs=4) as ep:
                for b in range(nbh):
                    gb = h * nbh + b
                    u = ep.tile([P, f_hid], f32, tag="u")
                    nc.vector.tensor_scalar(
                        out=u[:], in0=ps_h[:, b * f_hid:(b + 1) * f_hid],
                        scalar1=dinv_sb[:, gb:gb + 1], scalar2=None,
                        op0=Alu.mult)
                    v = ep.tile([P, f_hid], f32, tag="v")
                    nc.vector.tensor_tensor(
                        out=v[:], in0=u[:], in1=b1b_sb[:], op=Alu.add)
                    nc.vector.tensor_scalar(
                        out=h1s_sb[:, gb * f_hid:(gb + 1) * f_hid], in0=v[:],
                        scalar1=0.0, scalar2=dinv_sb[:, gb:gb + 1],
                        op0=Alu.max, op1=Alu.mult)

        with tc.tile_pool(name="aggps1", bufs=1, space="PSUM") as pspool1:
            emit_agg(xg1, f_hid, xt1_sb, epi1, pspool1)
        if dbg:
            nc.sync.dma_start(out=dbg_h1s[:], in_=h1s_sb[:])

        # ---- xt2 = h1s @ W2 (h1s already carries the dinv source scale)
        with tc.tile_pool(name="x2prep", bufs=3) as xp2, \
                tc.tile_pool(name="x2ps", bufs=2, space="PSUM") as xps2:
            for gb in range(nblk):
                w = min(P, ns - gb * P)
                if w <= 0:
                    break
                ps_hT = xps2.tile([f_hid, P], f32, tag="ps_hT")
                nc.tensor.transpose(
                    out=ps_hT[:, :w],
                    in_=h1s_sb[:w, gb * f_hid:(gb + 1) * f_hid],
                    identity=ident_sb[:w, :w])
                hT = xp2.tile([f_hid, P], f32, tag="hT")
                nc.vector.tensor_copy(out=hT[:, :w], in_=ps_hT[:, :w])
                ps_x2 = xps2.tile([f_out, P], f32, tag="ps_x2")
                nc.tensor.matmul(out=ps_x2[:, :w], lhsT=w2_sb[:],
                                 rhs=hT[:, :w], start=True, stop=True)
                x2T = xp2.tile([f_out, P], f32, tag="x2T")
                nc.vector.tensor_copy(out=x2T[:, :w], in_=ps_x2[:, :w])
                ps_t2 = xps2.tile([P, f_out], f32, tag="ps_t2")
                nc.tensor.transpose(out=ps_t2[:w, :], in_=x2T[:, :w],
                                    identity=ident_sb[:f_out, :f_out])
                nc.vector.tensor_copy(
                    out=xt2_sb[:w, gb * f_out:(gb + 1) * f_out],
                    in_=ps_t2[:w, :])
                nc.sync.dma_start(
                    out=xloc2[gb * P:gb * P + w, 0:f_out],
                    in_=xt2_sb[:w, gb * f_out:(gb + 1) * f_out])

        nc.gpsimd.collective_compute(
            "AllGather", Alu.bypass, replica_groups=groups,
            ins=[xloc2[:]], outs=[xg2[:]])

        # L2 epilogue: out = dinv*ps + b2 -> DRAM
        def epi2(h, ps_h):
            with tc.tile_pool(name="epi2", bufs=4) as ep:
                for b in range(nbh):
                    gb = h * nbh + b
                    u = ep.tile([P, f_out], f32, tag="u2")
                    nc.vector.tensor_scalar(
                        out=u[:], in0=ps_h[:, b * f_out:(b + 1) * f_out],
                        scalar1=dinv_sb[:, gb:gb + 1], scalar2=None,
                        op0=Alu.mult)
                    o = ep.tile([P, f_out], f32, tag="o2")
                    nc.vector.tensor_tensor(
                        out=o[:], in0=u[:], in1=b2b_sb[:], op=Alu.add)
                    nc.sync.dma_start(out=out_d[gb * P:(gb + 1) * P, :],
                                      in_=o[:])

        with tc.tile_pool(name="aggps2", bufs=1, space="PSUM") as pspool2:
            emit_agg(xg2, f_out, xt2_sb, epi2, pspool2)

    nc.compile()
    return nc


# ---------------------------------------------------------------- entry point

def _run(cfg: Cfg, z, edge_index, edge_attr, W1, b1, W2, b2, dbg=False):
    import ml_dtypes
    from concourse.bass_utils import run_bass_kernel_spmd

    import time as _time
    _t = _time.time()
    sched, per_core = preprocess(cfg, np.asarray(edge_index),
                                 np.asarray(edge_attr, dtype=np.float32))
    print(f"[kernel] preprocess {_time.time()-_t:.1f}s "
          f"tiles/layer={sched['total_tiles']}", flush=True)
    _t = _time.time()
    nc = build_program(cfg, sched, dbg=dbg)
    print(f"[kernel] build+schedule {_time.time()-_t:.1f}s", flush=True)

    z = np.asarray(z, dtype=np.float32)
    W1 = np.asarray(W1, dtype=np.float32)
    b1 = np.asarray(b1, dtype=np.float32)
    W2 = np.asarray(W2, dtype=np.float32)
    b2 = np.asarray(b2, dtype=np.float32)

    iota = np.tile(
        np.arange(P, dtype=np.float32).astype(ml_dtypes.bfloat16)[None, :],
        (P, KB))
    ident = np.eye(P, dtype=np.float32)
    identb = np.eye(P, dtype=np.float32).astype(ml_dtypes.bfloat16)
    b1b = np.tile(b1[None, :], (P, 1)).astype(np.float32)
    b2b = np.tile(b2[None, :], (P, 1)).astype(np.float32)

    in_maps = []
    for c in range(cfg.ncores):
        pc = per_core[c]
        zt = np.ascontiguousarray(
            z[c * cfg.nshard:(c + 1) * cfg.nshard, :].T)
        m = {
            "zT": zt, "W1": W1, "W2": W2, "b1b": b1b, "b2b": b2b,
            "iota": iota, "ident": ident, "identb": identb,
            "degpad": pc["degpad"],
            "dloc": pc["dloc_col"], "ew": pc["ew_col"],
        }
        for h in range(2):
            for qq in range(cfg.qn):
                m[f"idx_h{h}q{qq}"] = pc["idx_segs"][(h, qq)]
        in_maps.append(m)

    _t = _time.time()
    res = run_bass_kernel_spmd(
        nc, in_maps, core_ids=list(range(cfg.ncores)),
        trace=bool(int(__import__("os").environ.get("KERNEL_TRACE", "0"))))
    print(f"[kernel] compile+run {_time.time()-_t:.1f}s", flush=True)

    out = np.concatenate(
        [res.results[c]["out"][:cfg.nshard] for c in range(cfg.ncores)], axis=0)
    return out.astype(np.float32), res


def kernel(z, edge_index, edge_attr, W1, b1, W2, b2):
    out, _ = _run(FULL_CFG, z, edge_index, edge_attr, W1, b1, W2, b2)
    return out
